# revision 1
# baseline (speedup 1.0000x reference)
"""GQA attention (RoPE, full softmax) on 8 TRN2 NeuronCores.

Strategy: tensor-parallel over heads. Core g owns KV head g and Q heads
4g..4g+3 (one GQA group). Each core computes a partial output
y_g = concat_h(softmax(Q_h K^T) V) @ wo_h^T summed over its 4 local heads;
the host sums the 8 partials (the "all-reduce").

Device layouts (host prepares everything transposed so no on-chip transposes
are needed):
  - xT   [DIM, SEQ]      x transposed; c-tiles [128, :] feed matmul contraction.
  - wqkv [CT, 128, 768]  per c-tile: cols 0:512 = 4 Q heads (wq^T, RoPE-permuted
                         rows), 512:640 = wk^T (permuted), 640:768 = wv^T.
  - woT  [QH, 128, DIM]  per local head h: wo[:, head_cols]^T.
  - csn  [3, 128, SEQ]   plane 0: [cos;cos], plane 1: [-sin;sin] (plane 2: ones;
                         transposed, for the even/odd -> halves permutation).

RoPE trick: interleaved (even,odd) pairs are permuted to (first half, second
half) by permuting wq/wk ROWS on the host. Scores are invariant to any common
permutation of the head dim of Q and K, so nothing else changes.

Softmax: scores ~ N(0,1) (|s|max ~ 8 for this data), so exp() without
max-subtraction is safe in fp32 and mathematically identical to the reference.
Row sums (over k = partition dim) are accumulated on the DVE in two parity
chains and collapsed with one ones-vector matmul (on the PE directly for
chunk 0, which has no wo filler work); the normalization is applied to O^T
(per-q-column scale broadcast across partitions via partition_broadcast).

Matmuls run as float32r (TF32-like, 1 cycle/row at free-dim >= 256; ~3e-4
end-to-end rel err). The kt loop is software-pipelined: PV(kt-1) runs after
S(kt) so exp(kt-1) has a full iteration of latency cover, and the previous
chunk's output-projection matmuls are drip-fed in as PE filler.
"""

import numpy as np

import concourse.bass as bass
import concourse.mybir as mybir
import concourse.tile as tile
from concourse import bacc
from concourse.bass_utils import run_bass_kernel_spmd
from concourse.masks import make_identity

F32 = mybir.dt.float32
F32R = mybir.dt.float32r
EXP = mybir.ActivationFunctionType.Exp

DIM, N_HEADS, N_KV_HEADS, HEAD_DIM, SEQ = 4096, 32, 8, 128, 2048
CORES = 8
QH = N_HEADS // CORES  # q heads per core
CHS = 512              # q-chunk size (= max fp32 moving free dim = 1 PSUM bank)


def _rope_batch(nc, rp, outs, ts, cs, sn):
    """outs[i] = ts[i]*cs + swap_halves(ts[i])*sn for a whole chunk.

    HW cannot mix partition base offsets within a compute op, so the halves
    swap goes through SBUF->SBUF DMAs (address-based, crosses partitions
    freely) -- batched over all heads to two DMAs per chunk.
    """
    n = outs[0].shape[-1]
    m = len(outs)
    tsb = rp.tile([128, m, n], F32, tag="ropesb", bufs=1)
    sw = rp.tile([128, m, n], F32, tag="ropesw", bufs=1)
    for i, t in enumerate(ts):
        nc.vector.tensor_copy(out=tsb[:, i, :], in_=t)   # PSUM -> SBUF
    nc.sync.dma_start(out=sw[0:64], in_=tsb[64:128])
    nc.sync.dma_start(out=sw[64:128], in_=tsb[0:64])
    for i, out in enumerate(outs):
        t1 = rp.tile([128, n], F32, tag="rope1")
        t2 = rp.tile([128, n], F32, tag="rope2")
        nc.vector.tensor_mul(t1, tsb[:, i, :], cs)
        nc.vector.tensor_mul(t2, sw[:, i, :], sn)
        nc.vector.tensor_add(out, t1, t2)


def _body(tc, xT, wqkv, woT, csn, y, dim, seq, qh):
    nc = tc.nc
    CT = dim // 128   # contraction tiles (model dim)
    KT = seq // 128   # key tiles
    CH = seq // CHS   # q chunks
    QS = seq // 128   # q sub-tiles (phase 3)
    ECH = dim // 512  # output-column chunks
    HD = HEAD_DIM
    scale = HD ** -0.5

    with tc.tile_pool(name="persist", bufs=1) as persist:
        QT = [persist.tile([128, seq], F32R, name=f"qt{h}", tag=f"qt{h}") for h in range(qh)]
        KTs = persist.tile([128, seq], F32R, tag="kts")
        Vs = persist.tile([128, KT, HD], F32R, tag="vs")
        ones = persist.tile([128, 1], F32R, tag="ones")
        nc.sync.dma_start(out=ones, in_=csn[2, :, 0:1].bitcast(F32R))
        ident = persist.tile([128, 128], F32, tag="ident")
        make_identity(nc, ident)

        # ---------------- Phase 1: Q/K/V projections + RoPE ----------------
        with (
            tc.tile_pool(name="wq", bufs=1) as wqp,
            tc.tile_pool(name="csn", bufs=1) as csp,
            tc.tile_pool(name="xs", bufs=7) as xs,
            tc.tile_pool(name="rope", bufs=2) as rp,
            tc.tile_pool(name="p1ps", bufs=1, space="PSUM") as pps,
            tc.tile_pool(name="trps", bufs=2, space="PSUM") as tps,
        ):
            cs_t = csp.tile([128, seq], F32, tag="cs")
            sn_t = csp.tile([128, seq], F32, tag="sn")
            wq_s = wqp.tile([128, CT, (qh + 2) * HD], F32R)

            for j in range(CH):
                jsl = slice(j * CHS, (j + 1) * CHS)
                qps = [pps.tile([128, CHS], F32, name=f"qps{h}", tag=f"qps{h}") for h in range(qh)]
                kps = pps.tile([128, CHS], F32, tag="kps")
                vtps = pps.tile([128, CHS], F32, tag="vtps")
                for c in range(CT):
                    if j == 0:
                        # stream weights just ahead of first use so the first
                        # matmuls don't sit behind a 12MB weight preload
                        nc.sync.dma_start(
                            out=wq_s[:, c, :], in_=wqkv[c].bitcast(F32R)
                        )
                    xt = xs.tile([128, CHS], F32R)
                    nc.sync.dma_start(
                        out=xt, in_=xT[c * 128:(c + 1) * 128, jsl].bitcast(F32R)
                    )
                    xtr = xt
                    if j == 0 and c == min(4, CT - 1):
                        # csn loads off the startup critical path
                        nc.sync.dma_start(out=cs_t, in_=csn[0])
                        nc.sync.dma_start(out=sn_t, in_=csn[1])
                    nc.tensor.matmul(
                        vtps,
                        lhsT=wq_s[:, c, (qh + 1) * HD:(qh + 2) * HD],
                        rhs=xtr,
                        start=(c == 0),
                        stop=(c == CT - 1),
                    )
                    nc.tensor.matmul(
                        kps,
                        lhsT=wq_s[:, c, qh * HD:(qh + 1) * HD],
                        rhs=xtr,
                        start=(c == 0),
                        stop=(c == CT - 1),
                    )
                    for h in range(qh):
                        nc.tensor.matmul(
                            qps[h],
                            lhsT=wq_s[:, c, h * HD:(h + 1) * HD],
                            rhs=xtr,
                            start=(c == 0),
                            stop=(c == CT - 1),
                        )
                # V^T copy first so its PSUM bank frees earliest (next
                # chunk's first matmul targets it)
                vt_sb = rp.tile([128, CHS], F32, tag="vtsb")
                nc.vector.tensor_copy(out=vt_sb, in_=vtps)
                _rope_batch(
                    nc, rp,
                    [KTs[:, jsl]] + [QT[h][:, jsl] for h in range(qh)],
                    [kps] + qps,
                    cs_t[:, jsl], sn_t[:, jsl],
                )
                for b in range(CHS // HD):
                    trp = tps.tile([128, HD], F32, tag="trp")
                    nc.tensor.transpose(trp, vt_sb[:, b * HD:(b + 1) * HD], ident)
                    nc.vector.tensor_copy(
                        out=Vs[:, j * (CHS // HD) + b, :], in_=trp
                    )

        # ---------------- Phases 2+3, interleaved per q-chunk ----------------
        # Per chunk j: attention for all 4 heads (ACT-heavy), then that
        # chunk's output projection (PE/DMA-heavy). Interleaving lets the
        # scheduler fill exp() stalls with wo matmuls of the previous chunk.
        with (
            tc.tile_pool(name="ot", bufs=1) as otp,
            tc.tile_pool(name="pp", bufs=3) as pp,
            tc.tile_pool(name="sm", bufs=2) as sm,
            tc.tile_pool(name="wo", bufs=1) as wop,
            tc.tile_pool(name="ysb", bufs=2) as ysb,
            tc.tile_pool(name="sps", bufs=3, space="PSUM") as sps,
            tc.tile_pool(name="aps", bufs=2, space="PSUM") as aps,
            tc.tile_pool(name="yps", bufs=2, space="PSUM") as yps,
        ):
            OT = [otp.tile([128, seq], F32R, name=f"ot{h}", tag=f"ot{h}") for h in range(qh)]
            wo_r = wop.tile([128, qh, dim], F32R)
            for h in range(qh):
                nc.sync.dma_start(out=wo_r[:, h, :], in_=woT[h].bitcast(F32R))

            def wo_gen(j):
                """Output projection for chunk j; one yield per wo matmul so
                it can be drip-fed into the next chunk's attention loop to
                fill the PE during exp() latency. The last chunk streams its
                output DMAs per 512-column piece so the kernel-exit drain
                doesn't wait on one final 2MB row transfer."""
                fine = j == CH - 1
                for qs in range(j * (CHS // 128), (j + 1) * (CHS // 128)):
                    ystage = ysb.tile([128, dim], F32, tag="yt")
                    for e in range(ECH):
                        esl = slice(e * 512, (e + 1) * 512)
                        yp = yps.tile([128, 512], F32, tag="yp")
                        for h in range(qh):
                            nc.tensor.matmul(
                                yp,
                                lhsT=OT[h][:, qs * 128:(qs + 1) * 128],
                                rhs=wo_r[:, h, esl],
                                start=(h == 0),
                                stop=(h == qh - 1),
                            )
                            yield
                        nc.vector.tensor_copy(out=ystage[:, esl], in_=yp)
                        if fine:
                            nc.sync.dma_start(
                                out=y[qs * 128:(qs + 1) * 128, esl],
                                in_=ystage[:, esl],
                            )
                    if not fine:
                        nc.sync.dma_start(
                            out=y[qs * 128:(qs + 1) * 128, :], in_=ystage
                        )

            pending = []

            def drain_wo(n):
                while n > 0 and pending:
                    try:
                        next(pending[0])
                        n -= 1
                    except StopIteration:
                        pending.pop(0)

            for j in range(CH):
                jsl = slice(j * CHS, (j + 1) * CHS)
                for h in range(qh):
                    ops_t = aps.tile([128, CHS], F32, tag="ops")
                    sums = aps.tile([1, CHS], F32, tag="sums", bufs=1)
                    pacc = (
                        [
                            pp.tile([128, CHS], F32R, name=f"pacc{i}",
                                    tag=f"pacc{i}", bufs=2)
                            for i in range(2)
                        ]
                        if j != 0
                        else None
                    )
                    # software-pipelined by one kt: PV(kt-1) is emitted after
                    # S(kt), so exp(kt-1) has a full iteration of latency cover;
                    # wo matmuls of the previous chunk drip in as extra filler.
                    pts = [None] * KT
                    for kt in range(KT):
                        st = sps.tile([128, CHS], F32, tag="st")
                        nc.tensor.matmul(
                            st,
                            lhsT=KTs[:, kt * 128:(kt + 1) * 128],
                            rhs=QT[h][:, jsl],
                            start=True,
                            stop=True,
                        )
                        pt = pp.tile([128, CHS], F32R, tag="pt", bufs=4)
                        nc.scalar.activation(out=pt, in_=st, func=EXP, scale=scale)
                        pts[kt] = pt
                        if j == 0:
                            # chunk 0 has no wo filler work; the ones-matmul
                            # sums occupy the PE during exp latency instead
                            nc.tensor.matmul(
                                sums,
                                lhsT=ones,
                                rhs=pt,
                                start=(kt == 0),
                                stop=(kt == KT - 1),
                            )
                        else:
                            # row-sum accumulation on DVE, two parity chains
                            acc = pacc[kt % 2]
                            if kt < 2:
                                nc.vector.tensor_copy(out=acc, in_=pt)
                            else:
                                nc.vector.tensor_add(acc, acc, pt)
                        drain_wo(1)
                        if kt > 0:
                            nc.tensor.matmul(
                                ops_t,
                                lhsT=Vs[:, kt - 1, :],
                                rhs=pts[kt - 1],
                                start=(kt - 1 == 0),
                                stop=False,
                            )
                        drain_wo(1)
                    nc.tensor.matmul(
                        ops_t,
                        lhsT=Vs[:, KT - 1, :],
                        rhs=pts[KT - 1],
                        start=False,
                        stop=True,
                    )
                    if j != 0:
                        nc.vector.tensor_add(pacc[0], pacc[0], pacc[1])
                        nc.tensor.matmul(sums, lhsT=ones, rhs=pacc[0], start=True, stop=True)
                    ssb = sm.tile([1, CHS], F32, tag="ssb")
                    nc.vector.tensor_copy(out=ssb, in_=sums)
                    rec = sm.tile([1, CHS], F32, tag="rec")
                    nc.vector.reciprocal(rec, ssb)
                    rb = sm.tile([128, CHS], F32, tag="rb")
                    nc.gpsimd.partition_broadcast(rb, rec)
                    nc.vector.tensor_mul(OT[h][:, jsl], ops_t, rb)

                # queue this chunk's output projection; it interleaves into
                # the next chunk's attention (or drains at the end)
                pending.append(wo_gen(j))

            while pending:
                drain_wo(64)


def build_nc(dim=DIM, seq=SEQ, qh=QH):
    ct = dim // 128
    nc = bacc.Bacc("TRN2", target_bir_lowering=False, debug=False)
    xT = nc.dram_tensor("xT", [dim, seq], F32, kind="ExternalInput").ap()
    wqkv = nc.dram_tensor(
        "wqkv", [ct, 128, (qh + 2) * HEAD_DIM], F32, kind="ExternalInput"
    ).ap()
    woT = nc.dram_tensor("woT", [qh, HEAD_DIM, dim], F32, kind="ExternalInput").ap()
    csn = nc.dram_tensor("csn", [3, 128, seq], F32, kind="ExternalInput").ap()
    y = nc.dram_tensor("y", [seq, dim], F32, kind="ExternalOutput").ap()
    with tile.TileContext(nc) as tc:
        _body(tc, xT, wqkv, woT, csn, y, dim, seq, qh)
    nc.compile()
    return nc


def make_in_maps(x, freqs, wq, wk, wv, wo, cores=CORES):
    """Host-side sharding: returns list of per-core input dicts."""
    dim = x.shape[1]
    seq = x.shape[0]
    hd = HEAD_DIM
    n_heads = wq.shape[0] // hd
    n_kv = wk.shape[0] // hd
    qh = n_heads // cores
    ct = dim // 128

    perm = np.concatenate([np.arange(0, hd, 2), np.arange(1, hd, 2)])
    cos = np.cos(freqs).T.astype(np.float32)  # [64, S]
    sin = np.sin(freqs).T.astype(np.float32)
    csn = np.stack(
        [
            np.concatenate([cos, cos], axis=0),
            np.concatenate([-sin, sin], axis=0),
            np.ones((128, seq), np.float32),
        ]
    ).astype(np.float32)  # [3, 128, S]

    xT = np.ascontiguousarray(x.T.astype(np.float32))  # [dim, seq]

    wq_r = wq.reshape(n_heads, hd, dim)
    wk_r = wk.reshape(n_kv, hd, dim)
    wv_r = wv.reshape(n_kv, hd, dim)

    in_maps = []
    for g in range(cores):
        wq_g = wq_r[g * qh:(g + 1) * qh][:, perm, :]  # [qh, 128, dim]
        wk_g = wk_r[g][perm, :]                       # [128, dim]
        wv_g = wv_r[g]                                # [128, dim]
        # -> [ct, 128, qh*128]: c-tile-major, transposed blocks
        wq_t = (
            wq_g.reshape(qh, hd, ct, 128).transpose(2, 3, 0, 1).reshape(ct, 128, qh * hd)
        )
        wk_t = wk_g.reshape(hd, ct, 128).transpose(1, 2, 0)  # [ct, 128, 128]
        wv_t = wv_g.reshape(hd, ct, 128).transpose(1, 2, 0)
        wqkv_g = np.ascontiguousarray(
            np.concatenate([wq_t, wk_t, wv_t], axis=2), dtype=np.float32
        )
        wo_g = wo[:, g * qh * hd:(g + 1) * qh * hd]   # [dim, qh*128]
        woT_g = np.ascontiguousarray(wo_g.T.reshape(qh, hd, dim), dtype=np.float32)
        in_maps.append({"xT": xT, "wqkv": wqkv_g, "woT": woT_g, "csn": csn})
    return in_maps


_NC_CACHE = {}


def kernel(x, freqs, wq, wk, wv, wo):
    x = np.asarray(x, dtype=np.float32)
    freqs = np.asarray(freqs, dtype=np.float32)
    wq = np.asarray(wq, dtype=np.float32)
    wk = np.asarray(wk, dtype=np.float32)
    wv = np.asarray(wv, dtype=np.float32)
    wo = np.asarray(wo, dtype=np.float32)

    key = (DIM, SEQ, QH)
    if key not in _NC_CACHE:
        _NC_CACHE[key] = build_nc(DIM, SEQ, QH)
    nc = _NC_CACHE[key]

    in_maps = make_in_maps(x, freqs, wq, wk, wv, wo, CORES)
    res = run_bass_kernel_spmd(nc, in_maps, list(range(CORES)))
    parts = [res.results[g]["y"] for g in range(CORES)]
    return np.sum(np.stack(parts), axis=0, dtype=np.float32)


if __name__ == "__main__":
    import reference

    inputs = reference.setup_inputs()
    out = kernel(**{k: np.asarray(v) for k, v in inputs.items()})
    print("kernel out", out.shape, out.dtype)

